# revision 21
# baseline (speedup 1.0000x reference)
"""Trainium2 Bass kernel for nn_EquivariantNodeFFN (equivariant gated FFN).

Strategy (pure data parallel over nodes, 8 cores x 8192 nodes):
  - Per core, process 8 pairs of 1024 nodes (8 subtiles of 128 each).
  - One input DMA per pair ([128, 8, 480] f32, node-major) and one output
    DMA per pair ([128, 4, 1024] bf16 -> o[512, 8192] feature-major; host
    adds the residual, converts to f32 and un-permutes columns).
  - Software-pipelined loop: pair b+2's input DMA and pair b+1's
    stats+casts are emitted before pair b's compute so per-engine queues
    always hold ready work.
  - Norm stats on-chip: l0 via DVE bn_stats (batched eps-add), l1/l2
    square-sums on ACT (Square + accum_out, same "silu_and_others" act
    table set); rsqrt via Quake seed (DVE int ops) + 2 Newton steps on
    Pool.
  - Normalization scales fused into the fp32->bf16 casts on DVE
    (tensor_scalar with per-partition scalar pointers; 2x perf mode).
  - PE transposes (identity matmul, tile_position packing) produce
    feature-major bf16 activations in [128, 1024] psum tiles (single
    rotating psum slot), drained by DVE (bf16 2x mode).
  - lin1/lin2 as 512-col matmuls writing into [128, 1024] f32 psum pair
    tiles; activations (silu + tanh for sigmoid-gating, 0.5 folded into
    V1/V2) run 1024 wide on ACT; gating runs 1024 wide on DVE
    (scalar_tensor_tensor (tg + 1) * h -> bf16).
  - lin2 output drains on ACT (o0 with c0 bias fused), 512-col psum tiles
    into the bf16 output buffer.
"""

import os
import sys

sys.path.insert(0, "/opt/trn_rl_repo")

import numpy as np
import ml_dtypes

import concourse.bass as bass
import concourse.bacc as bacc
import concourse.tile as tile
from concourse import mybir
from concourse.bass_utils import run_bass_kernel_spmd

F32 = mybir.dt.float32
BF16 = mybir.dt.bfloat16
I32 = mybir.dt.int32
AF = mybir.ActivationFunctionType
OP = mybir.AluOpType

# ---- problem constants (hardcoded per contract) ----
N_NODES = 65536
N_CORES = 8
NC = N_NODES // N_CORES      # 8192 nodes per core
PAIR = 1024                  # nodes per pair-block
NSUB = 8                     # subtiles of 128 per pair
SUB = 128
NPAIR = NC // PAIR           # 8

M0, M1, M2 = 128, 64, 32
H0, H1, H2 = 512, 256, 128
G = H1 + H2                  # 384
D_IN = M0 + 3 * M1 + 5 * M2  # 480
EPS = 1e-8
S0, S1, S2 = float(np.sqrt(M0)), float(np.sqrt(M1)), float(np.sqrt(M2))
T0, T1, T2 = float(np.sqrt(H0)), float(np.sqrt(H1)), float(np.sqrt(H2))

MAGIC = 0x5F3759DF

# device feature-row order (m-major within each degree) -> x column
PERM = np.array(
    list(range(M0))
    + [M0 + 3 * u + m for m in range(3) for u in range(M1)]
    + [M0 + 3 * M1 + 5 * u + m for m in range(5) for u in range(M2)]
)

_BUILT = None
TRACE = False
TRACE_KW = {}
LAST_RESULTS = None
SQ_ON_ACT = os.environ.get("SQ_ON_ACT", "1") == "1"
TDRAIN_ON_ACT = os.environ.get("TDRAIN_ON_ACT", "0") == "1"
L12CAST_ON_ACT = os.environ.get("L12CAST_ON_ACT", "0") == "1"


def _build_bass(nrep=1):
    nc = bacc.Bacc("TRN2", target_bir_lowering=False)

    x_d = nc.dram_tensor("x", [NC, D_IN], F32, kind="ExternalInput")
    w0_d = nc.dram_tensor("w0", [128, 7, 128], BF16, kind="ExternalInput")
    w1_d = nc.dram_tensor("w1", [128, 2, 128], BF16, kind="ExternalInput")
    w2_d = nc.dram_tensor("w2", [128, 128], BF16, kind="ExternalInput")
    v0_d = nc.dram_tensor("v0", [128, 4, 128], BF16, kind="ExternalInput")
    v1_d = nc.dram_tensor("v1", [128, 2, 64], BF16, kind="ExternalInput")
    v2_d = nc.dram_tensor("v2", [128, 32], BF16, kind="ExternalInput")
    b0_d = nc.dram_tensor("b0", [128, 7], F32, kind="ExternalInput")
    c0_d = nc.dram_tensor("c0", [128, 1], F32, kind="ExternalInput")
    eye_d = nc.dram_tensor("eye", [128, 128], BF16, kind="ExternalInput")
    o_d = nc.dram_tensor("o", [512, NC], BF16, kind="ExternalOutput")

    with tile.TileContext(nc) as tc:
        with (
            tc.tile_pool(name="const", bufs=1) as const,
            tc.tile_pool(name="xin", bufs=3) as xin,
            tc.tile_pool(name="xb", bufs=18) as xbp,
            tc.tile_pool(name="stat", bufs=10) as statp,
            tc.tile_pool(name="bstat", bufs=2) as bstat,
            tc.tile_pool(name="ysb", bufs=2) as ysb,
            tc.tile_pool(name="act", bufs=5) as actp,
            tc.tile_pool(name="z", bufs=14) as zp,
            tc.tile_pool(name="ob", bufs=2) as obp_pool,
            tc.tile_pool(name="dump", bufs=2) as dumpp,
            tc.tile_pool(name="tp", bufs=2, space="PSUM") as tpp,
            tc.tile_pool(name="hp", bufs=2, space="PSUM") as hpp,
            tc.tile_pool(name="op", bufs=1, space="PSUM") as opp,
        ):
            # constants
            w0s = const.tile([128, 7, 128], BF16)
            w1s = const.tile([128, 2, 128], BF16)
            w2s = const.tile([128, 128], BF16)
            v0s = const.tile([128, 4, 128], BF16)
            v1s = const.tile([128, 2, 64], BF16)
            v2s = const.tile([128, 32], BF16)
            b0s = const.tile([128, 7], F32)
            c0s = const.tile([128, 1], F32)
            eye = const.tile([128, 128], BF16)
            magic = const.tile([128, 16], I32)
            cneg = const.tile([128, 16], F32)   # -0.5
            c15 = const.tile([128, 16], F32)    # 1.5
            keps = const.tile([128, 16], F32)   # EPS
            k384 = const.tile([128, 8], F32)    # 1/384
            k320 = const.tile([128, 8], F32)    # 1/320
            for sb, dr in ((w0s, w0_d), (w1s, w1_d), (w2s, w2_d), (v0s, v0_d),
                           (v1s, v1_d), (v2s, v2_d), (b0s, b0_d), (c0s, c0_d),
                           (eye, eye_d)):
                nc.sync.dma_start(out=sb[:], in_=dr[:])
            nc.vector.memset(magic[:], MAGIC)
            nc.gpsimd.memset(cneg[:], -0.5)
            nc.gpsimd.memset(c15[:], 1.5)
            nc.gpsimd.memset(keps[:], EPS)
            nc.gpsimd.memset(k384[:], 1.0 / 384.0)
            nc.gpsimd.memset(k320[:], 1.0 / 320.0)

            NB = NPAIR * nrep

            def dma_phase(b):
                j0 = (b % NPAIR) * PAIR
                xt = xin.tile([128, NSUB, D_IN], F32, tag="x")
                nc.sync.dma_start(
                    out=xt[:],
                    in_=x_d[j0:j0 + PAIR, :].rearrange("(s p) c -> p s c", p=128))
                return xt

            def stats_phase(xt):
                # --- per-subtile stats ---
                vq = bstat.tile([128, 16], F32, tag="vq")      # 0:8 v0+eps, 8:16 q
                sq1c = bstat.tile([128, 8], F32, tag="sq1")
                sq2c = bstat.tile([128, 8], F32, tag="sq2")
                tmp8 = bstat.tile([128, 8], F32, tag="tmp8")
                ynt = bstat.tile([128, 16], F32, tag="ynt")    # 0:8 rstd, 8:16 inv
                yi32 = ynt[:].bitcast(I32)
                aux = bstat.tile([128, 16], F32, tag="aux")
                aux2 = bstat.tile([128, 16], F32, tag="aux2")
                dump = dumpp.tile([128, 352], F32, tag="d")

                mvall = bstat.tile([128, 2 * NSUB], F32, tag="mva")
                for s in range(NSUB):
                    st6 = statp.tile([128, 6], F32, tag="st6")
                    nc.vector.bn_stats(out=st6[:], in_=xt[:, s, 0:M0])
                    nc.vector.bn_aggr(out=mvall[:, 2 * s:2 * s + 2], in_=st6[:])
                    if SQ_ON_ACT:
                        nc.scalar.activation(
                            out=dump[:, 0:192], in_=xt[:, s, 128:320],
                            func=AF.Square, accum_out=sq1c[:, s:s + 1])
                        nc.scalar.activation(
                            out=dump[:, 192:352], in_=xt[:, s, 320:480],
                            func=AF.Square, accum_out=sq2c[:, s:s + 1])
                    else:
                        nc.vector.scalar_tensor_tensor(
                            out=dump[:, 0:192], in0=xt[:, s, 128:320], scalar=1.0,
                            in1=xt[:, s, 128:320], op0=OP.mult, op1=OP.mult,
                            accum_out=sq1c[:, s:s + 1])
                        nc.vector.scalar_tensor_tensor(
                            out=dump[:, 192:352], in0=xt[:, s, 320:480], scalar=1.0,
                            in1=xt[:, s, 320:480], op0=OP.mult, op1=OP.mult,
                            accum_out=sq2c[:, s:s + 1])
                # one batched eps-add over the 8 strided var columns
                mvr = mvall[:].rearrange("p (s c) -> p c s", c=2)
                nc.gpsimd.tensor_tensor(
                    out=vq[:, 0:8], in0=mvr[:, 1, :], in1=keps[:, 0:8], op=OP.add)

                # --- combine + rsqrt (quake seed on DVE, newton on Pool) ---
                nc.gpsimd.tensor_tensor(out=tmp8[:], in0=sq2c[:], in1=k320[:], op=OP.mult)
                nc.gpsimd.tensor_tensor(out=sq1c[:], in0=sq1c[:], in1=k384[:], op=OP.mult)
                nc.gpsimd.tensor_tensor(out=tmp8[:], in0=tmp8[:], in1=sq1c[:], op=OP.add)
                nc.gpsimd.tensor_tensor(out=vq[:, 8:16], in0=tmp8[:], in1=keps[:, 0:8], op=OP.add)

                vi32 = vq[:].bitcast(I32)
                nc.vector.tensor_scalar(
                    out=yi32, in0=vi32, scalar1=1, scalar2=None,
                    op0=OP.arith_shift_right)
                nc.vector.scalar_tensor_tensor(
                    out=yi32, in0=magic[:], scalar=0, in1=yi32,
                    op0=OP.bypass, op1=OP.subtract)
                for _ in range(2):
                    nc.gpsimd.tensor_tensor(out=aux[:], in0=ynt[:], in1=ynt[:], op=OP.mult)
                    nc.gpsimd.tensor_tensor(out=aux2[:], in0=aux[:], in1=vq[:], op=OP.mult)
                    nc.gpsimd.tensor_tensor(out=aux2[:], in0=aux2[:], in1=cneg[:], op=OP.mult)
                    nc.gpsimd.tensor_tensor(out=aux[:], in0=aux2[:], in1=c15[:], op=OP.add)
                    nc.gpsimd.tensor_tensor(out=ynt[:], in0=ynt[:], in1=aux[:], op=OP.mult)
                # ynt cols 0:8 = rstd(s), 8:16 = inv(s)

                # --- normalize + cast (l0 on DVE, l1/l2 on ACT) ---
                xc_s = []
                xb_s = []
                for s in range(NSUB):
                    xc = xbp.tile([128, 128], BF16, tag="xc")
                    nc.vector.tensor_scalar(
                        out=xc[:], in0=xt[:, s, 0:M0], scalar1=mvall[:, 2 * s:2 * s + 1],
                        scalar2=ynt[:, s:s + 1], op0=OP.subtract, op1=OP.mult)
                    xc_s.append(xc)
                    xb = xbp.tile([128, 352], BF16, tag="xb")
                    if L12CAST_ON_ACT:
                        nc.scalar.mul(out=xb[:], in_=xt[:, s, 128:480],
                                      mul=ynt[:, 8 + s:9 + s])
                    else:
                        nc.vector.tensor_scalar(
                            out=xb[:], in0=xt[:, s, 128:480],
                            scalar1=ynt[:, 8 + s:9 + s], scalar2=None, op0=OP.mult)
                    xb_s.append(xb)
                return xc_s, xb_s

            def compute_phase(b, tiles):
                j0 = (b % NPAIR) * PAIR
                xc_s, xb_s = tiles
                # --- transposes to feature-major (PE), one rotating [128, 1024]
                # bf16 psum tile per target, drained to sbuf right away ---
                y0t = ysb.tile([128, PAIR], BF16, tag="y0")
                t1t = ysb.tile([128, PAIR], BF16, tag="t1")
                t2t = ysb.tile([128, PAIR], BF16, tag="t2")
                t3t = ysb.tile([96, PAIR], BF16, tag="t3")
                l1 = [xb_s[s][:, 0:192].rearrange("p (u m) -> p m u", m=3)
                      for s in range(NSUB)]
                l2 = [xb_s[s][:, 192:352].rearrange("p (u m) -> p m u", m=5)
                      for s in range(NSUB)]

                tp = tpp.tile([128, PAIR], BF16, tag="tp")
                for s in range(NSUB):
                    sc = slice(s * SUB, (s + 1) * SUB)
                    nc.tensor.transpose(out=tp[:, sc], in_=xc_s[s][:], identity=eye[:])
                (nc.scalar.copy if TDRAIN_ON_ACT else nc.vector.tensor_copy)(out=y0t[:], in_=tp[:])

                tp = tpp.tile([128, PAIR], BF16, tag="tp")
                for s in range(NSUB):
                    sc = slice(s * SUB, (s + 1) * SUB)
                    nc.tensor.transpose(out=tp[0:64, sc], in_=l1[s][:, 0, :], identity=eye[:],
                                        tile_position=(0, 0))
                    nc.tensor.transpose(out=tp[64:128, sc], in_=l1[s][:, 1, :], identity=eye[:],
                                        tile_position=(0, 64))
                (nc.scalar.copy if TDRAIN_ON_ACT else nc.vector.tensor_copy)(out=t1t[:], in_=tp[:])

                tp = tpp.tile([128, PAIR], BF16, tag="tp")
                for s in range(NSUB):
                    sc = slice(s * SUB, (s + 1) * SUB)
                    nc.tensor.transpose(out=tp[0:64, sc], in_=l1[s][:, 2, :], identity=eye[:],
                                        tile_position=(0, 0))
                    nc.tensor.transpose(out=tp[64:96, sc], in_=l2[s][:, 0, :], identity=eye[:],
                                        tile_position=(0, 64))
                    nc.tensor.transpose(out=tp[96:128, sc], in_=l2[s][:, 1, :], identity=eye[:],
                                        tile_position=(0, 96))
                (nc.scalar.copy if TDRAIN_ON_ACT else nc.vector.tensor_copy)(out=t2t[:], in_=tp[:])

                tp = tpp.tile([128, PAIR], BF16, tag="tp")
                for s in range(NSUB):
                    sc = slice(s * SUB, (s + 1) * SUB)
                    nc.tensor.transpose(out=tp[0:32, sc], in_=l2[s][:, 2, :], identity=eye[:],
                                        tile_position=(0, 0))
                    nc.tensor.transpose(out=tp[32:64, sc], in_=l2[s][:, 3, :], identity=eye[:],
                                        tile_position=(0, 32))
                    nc.tensor.transpose(out=tp[64:96, sc], in_=l2[s][:, 4, :], identity=eye[:],
                                        tile_position=(0, 64))
                (nc.scalar.copy if TDRAIN_ON_ACT else nc.vector.tensor_copy)(out=t3t[:], in_=tp[0:96, :])

                rhs1 = [t1t[0:64, :], t1t[64:128, :], t2t[0:64, :]]
                rhs2 = [t2t[64:96, :], t2t[96:128, :], t3t[0:32, :],
                        t3t[32:64, :], t3t[64:96, :]]

                halves = (slice(0, 512), slice(512, 1024))

                # --- lin1 l0 + activations (1024 wide) ---
                s_sb = []
                tg_sb = []
                for c in range(7):
                    h0p = hpp.tile([128, PAIR], F32, tag="h")
                    for hv in halves:
                        nc.tensor.matmul(h0p[:, hv], w0s[:, c, :], y0t[:, hv],
                                         start=True, stop=True)
                    if c < 4:
                        st = actp.tile([128, PAIR], BF16, tag="s")
                        nc.scalar.activation(out=st[:], in_=h0p[:], func=AF.Silu,
                                             bias=b0s[:, c:c + 1], scale=1.0)
                        s_sb.append(st)
                    else:
                        tg = actp.tile([128, PAIR], F32, tag="tg")
                        nc.scalar.activation(out=tg[:], in_=h0p[:], func=AF.Tanh,
                                             bias=b0s[:, c:c + 1], scale=0.5)
                        tg_sb.append(tg)

                # --- lin1 l1/l2 + gating (1024 wide) ---
                z1_sb = [[None] * 3 for _ in range(2)]
                for c in range(2):
                    for m in range(3):
                        h1p = hpp.tile([128, PAIR], F32, tag="h")
                        base = 0 if m != 1 else 64
                        for hv in halves:
                            nc.tensor.matmul(h1p[:, hv], w1s[base:base + 64, c, :],
                                             rhs1[m][:, hv], start=True, stop=True,
                                             tile_position=(base, 0))
                        zt = zp.tile([128, PAIR], BF16, tag="z")
                        nc.vector.scalar_tensor_tensor(
                            out=zt[:], in0=tg_sb[c][:], scalar=1.0, in1=h1p[:],
                            op0=OP.add, op1=OP.mult)
                        z1_sb[c][m] = zt
                z2_sb = []
                for m in range(5):
                    h2p = hpp.tile([128, PAIR], F32, tag="h")
                    base = [64, 96, 0, 32, 64][m]
                    for hv in halves:
                        nc.tensor.matmul(h2p[:, hv], w2s[base:base + 32, :],
                                         rhs2[m][:, hv], start=True, stop=True,
                                         tile_position=(base, 0))
                    zt = zp.tile([128, PAIR], BF16, tag="z")
                    nc.vector.scalar_tensor_tensor(
                        out=zt[:], in0=tg_sb[2][:], scalar=1.0, in1=h2p[:],
                        op0=OP.add, op1=OP.mult)
                    z2_sb.append(zt)

                # --- lin2 (feature-major out, m-major rows), per 512 half ---
                obuf = obp_pool.tile([128, 4, PAIR], BF16, tag="ob")
                o0p = opp.tile([128, PAIR], F32, tag="o")
                for hv in halves:
                    for k in range(4):
                        nc.tensor.matmul(o0p[:, hv], v0s[:, k, :], s_sb[k][:, hv],
                                         start=(k == 0), stop=(k == 3))
                nc.scalar.activation(out=obuf[:, 0, :], in_=o0p[:],
                                     func=AF.Identity, bias=c0s[:, 0:1],
                                     scale=1.0)
                oap = opp.tile([128, PAIR], F32, tag="o")
                for hv in halves:
                    for m in range(2):
                        for k in range(2):
                            nc.tensor.matmul(oap[m * 64:(m + 1) * 64, hv], v1s[:, k, :],
                                             z1_sb[k][m][:, hv], start=(k == 0),
                                             stop=(k == 1), tile_position=(0, m * 64))
                nc.scalar.copy(out=obuf[:, 1, :], in_=oap[:])
                obp = opp.tile([128, PAIR], F32, tag="o")
                for hv in halves:
                    for k in range(2):
                        nc.tensor.matmul(obp[0:64, hv], v1s[:, k, :], z1_sb[k][2][:, hv],
                                         start=(k == 0), stop=(k == 1),
                                         tile_position=(0, 0))
                    nc.tensor.matmul(obp[64:96, hv], v2s[:], z2_sb[0][:, hv], start=True,
                                     stop=True, tile_position=(0, 64))
                    nc.tensor.matmul(obp[96:128, hv], v2s[:], z2_sb[1][:, hv], start=True,
                                     stop=True, tile_position=(0, 96))
                nc.scalar.copy(out=obuf[:, 2, :], in_=obp[:])
                ocp = opp.tile([128, PAIR], F32, tag="o")
                for hv in halves:
                    for m in range(3):
                        nc.tensor.matmul(ocp[m * 32:(m + 1) * 32, hv], v2s[:],
                                         z2_sb[2 + m][:, hv], start=True, stop=True,
                                         tile_position=(0, m * 32))
                nc.scalar.copy(out=obuf[0:96, 3, :], in_=ocp[0:96, :])

                nc.sync.dma_start(
                    out=o_d[:, j0:j0 + PAIR].rearrange("(g p) n -> p g n", p=128),
                    in_=obuf[:])

            xts = {}
            tiles = {}
            xts[0] = dma_phase(0)
            xts[1] = dma_phase(1)
            tiles[0] = stats_phase(xts.pop(0))
            for b in range(NB):
                if b + 2 < NB:
                    xts[b + 2] = dma_phase(b + 2)
                if b + 1 < NB:
                    tiles[b + 1] = stats_phase(xts.pop(b + 1))
                compute_phase(b, tiles.pop(b))

    nc.finalize()
    return nc


def _host_weights(inputs):
    bf = ml_dtypes.bfloat16
    t = float(np.tanh(np.float32(inputs["alpha"])))
    nw0 = np.asarray(inputs["nw0"], np.float32)
    nb0 = np.asarray(inputs["nb0"], np.float32)
    nw1 = np.asarray(inputs["nw1"], np.float32)
    nw2 = np.asarray(inputs["nw2"], np.float32)
    W0 = np.asarray(inputs["W0"], np.float32)
    W1 = np.asarray(inputs["W1"], np.float32)
    W2 = np.asarray(inputs["W2"], np.float32)
    V0 = np.asarray(inputs["V0"], np.float32)
    V1 = np.asarray(inputs["V1"], np.float32)
    V2 = np.asarray(inputs["V2"], np.float32)
    b0 = np.asarray(inputs["b0"], np.float32)
    c0 = np.asarray(inputs["c0"], np.float32)

    W0eff = (nw0[:, None] * W0) / S0                      # [128, 896]
    b0eff = b0 + (nb0 @ W0) / S0                          # [896]
    b0act = b0eff.copy()
    b0act[H0:] *= 0.5
    W1eff = (nw1[:, None] * W1) / S1                      # [64, 256]
    W2eff = (nw2[:, None] * W2) / S2                      # [32, 128]
    V0eff = t * V0 / T0                                   # [512, 128]
    V1eff = 0.5 * t * V1 / T1                             # [256, 64]
    V2eff = 0.5 * t * V2 / T2                             # [128, 32]
    c0eff = t * c0                                        # [128]

    w0 = np.ascontiguousarray(W0eff.reshape(128, 7, 128), dtype=bf)
    w1c = np.stack([W1eff[:, 0:128], W1eff[:, 128:256]], axis=1)  # [64, 2, 128]
    w1 = np.ascontiguousarray(np.concatenate([w1c, w1c], axis=0), dtype=bf)
    w2 = np.ascontiguousarray(np.concatenate([W2eff] * 4, axis=0), dtype=bf)  # [128,128]
    v0 = np.ascontiguousarray(
        V0eff.reshape(4, 128, 128).transpose(1, 0, 2), dtype=bf)  # [128,4,128]
    v1 = np.ascontiguousarray(V1eff.reshape(2, 128, 64).transpose(1, 0, 2), dtype=bf)
    v2 = np.ascontiguousarray(V2eff, dtype=bf)
    b0t = np.ascontiguousarray(b0act.reshape(7, 128).T, dtype=np.float32)  # [128,7]
    c0t = np.ascontiguousarray(c0eff.reshape(128, 1), dtype=np.float32)
    eye = np.ascontiguousarray(np.eye(128), dtype=bf)
    return dict(w0=w0, w1=w1, w2=w2, v0=v0, v1=v1, v2=v2, b0=b0t, c0=c0t, eye=eye)


def kernel(**inputs):
    global _BUILT
    if _BUILT is None:
        _BUILT = _build_bass()
    nc = _BUILT

    x = np.ascontiguousarray(np.asarray(inputs["x"], np.float32))
    wd = _host_weights(inputs)
    in_maps = []
    for c in range(N_CORES):
        m = {"x": np.ascontiguousarray(x[c * NC:(c + 1) * NC, :])}
        m.update(wd)
        in_maps.append(m)

    global LAST_RESULTS
    res = run_bass_kernel_spmd(nc, in_maps, core_ids=list(range(N_CORES)),
                               trace=TRACE, **TRACE_KW)
    LAST_RESULTS = res

    out = np.empty((N_NODES, D_IN), np.float32)
    for c in range(N_CORES):
        o_c = np.asarray(res.results[c]["o"])[:D_IN]     # [480, 8192] bf16 = t*dx
        oc = np.empty((NC, D_IN), np.float32)
        oc[:, PERM] = o_c.T.astype(np.float32)
        out[c * NC:(c + 1) * NC, :] = oc
    out += x
    return out


if __name__ == "__main__":
    ins = {k: np.asarray(v) for k, v in np.load(sys.argv[1], allow_pickle=True).item().items()}
    kernel(**ins)
